# revision 35
# baseline (speedup 1.0000x reference)
"""Multi-head attention (strictly-future mask) on 8 TRN2 cores — v2.

Reference math (B=4, S=2048, D=512, H=8, A=64):
    q/k/v = per-head projections                              [B,H,S,A]
    scores = q @ k^T / 8, lower triangle (k <= q) masked to -1e9
    out = concat_heads(softmax(scores) @ v) @ Wo + bo         [B,S,D]

Sharding: head-parallel within a batch — core c = (batch b = c//2,
head-half hp = c%2).  Each core computes 4 heads (= 2 stacked head
pairs) over the FULL 2048-query range, producing a partial output
summed on the host (Wo split along its input axis, per the TP hint);
host reduction replaces the all-reduce.

Bias algebra: softmax cancels any per-query additive score term, so
bk is dropped (its q-dependent term cancels) and only bq is kept on Q
(it produces the surviving per-key term).  bv contributes exactly
bv @ Wo_head (softmax weights sum to 1) — folded with bo into a host
constant.  K/V psum evictions are therefore pure copies.

Device dataflow (per core):
  * Q/K projections (transposed [a,S] layout) and V projection
    (natural [k,a] layout, input as stationary operand — no PE
    transposes needed) run as fp8e4 DoubleRow matmuls: contraction
    512 = 2x(2x128) chunk pairs at 0.5 cycles/column.
  * Scores are computed transposed (S^T[k,q]) in bf16; per head-pair
    strips kc carry a uniform pair width w_j = 256(j+1) so DoubleRow
    can pair even/odd k-strips in the AV stage.  exp() on the scalar
    engine writes P^T directly as fp8e4; diagonal blocks are masked
    post-exp by a 0/1 tril multiply; the even strip's overhang block
    is never computed — just zero-filled.
  * AV runs as fp8 DoubleRow over strip pairs with the [V|ones|V]
    stationary trick replicating softmax denominators; normalization
    divides via copy + reciprocal_approx_fast on DVE.
  * Output projection consumes X^T per 128-query tile, producing the
    natural [2048, 512] f32 partial output.

The last 32 query rows (tiny attention fan-in, where fp8 quantization
noise is largest — and q = S-1 is 0/0) are recomputed exactly on the
host during the gather.
"""

import numpy as np
import ml_dtypes

B, S, D, H, A = 4, 2048, 512, 8, 64
P = 128
NPAIR = 2            # head pairs per core
NHEAD = 4            # heads per core
NJ = 8               # strip pairs per head
FIX_ROWS = 64        # host-recomputed tail rows
BF = ml_dtypes.bfloat16
F8 = ml_dtypes.float8_e4m3

W_J = [256 * (j + 1) for j in range(NJ)]   # uniform pair strip widths

_cache = {}


def _chunks(lo, hi, step):
    out = []
    while lo < hi:
        out.append((lo, min(hi, lo + step)))
        lo = out[-1][1]
    return out


def _build():
    if "nc" in _cache:
        return _cache["nc"]

    import concourse.bacc as bacc
    import concourse.mybir as mybir
    import concourse.tile as tile

    F32 = mybir.dt.float32
    BF16 = mybir.dt.bfloat16
    FP8 = mybir.dt.float8e4
    MULT = mybir.AluOpType.mult
    ADD = mybir.AluOpType.add
    EXP = mybir.ActivationFunctionType.Exp
    DR = mybir.MatmulPerfMode.DoubleRow

    nc = bacc.Bacc("TRN2", target_bir_lowering=False, debug=False, num_devices=8)

    xq_d = nc.dram_tensor("xq", [P, 4 * S], FP8, kind="ExternalInput")
    xk_d = nc.dram_tensor("xk", [P, 4 * S], FP8, kind="ExternalInput")
    xv_d = nc.dram_tensor("xv", [P, 4 * S], FP8, kind="ExternalInput")
    wq_d = nc.dram_tensor("wq", [P, 1024], FP8, kind="ExternalInput")
    wk_d = nc.dram_tensor("wk", [P, 1024], FP8, kind="ExternalInput")
    wv_d = nc.dram_tensor("wv", [P, 1024], FP8, kind="ExternalInput")
    wo_d = nc.dram_tensor("wo", [P, 1024], BF16, kind="ExternalInput")
    bq_d = nc.dram_tensor("bq", [P, 2], F32, kind="ExternalInput")
    mk_d = nc.dram_tensor("mask", [P, P], BF16, kind="ExternalInput")
    out_d = nc.dram_tensor("out", [S, D], BF16, kind="ExternalOutput")

    with tile.TileContext(nc) as tc:
        with (
            tc.tile_pool(name="cst", bufs=1) as cst,
            tc.tile_pool(name="act", bufs=1) as act,
            tc.tile_pool(name="pts", bufs=3) as pts,
            tc.tile_pool(name="rcp", bufs=4) as rcp,
            tc.tile_pool(name="ost", bufs=3) as ost,
            tc.tile_pool(name="stg", bufs=3, space="PSUM") as stg,
            tc.tile_pool(name="avp", bufs=2, space="PSUM") as avp,
        ):
            xq = cst.tile([P, 4 * S], FP8, tag="xq")
            xk = cst.tile([P, 4 * S], FP8, tag="xk")
            xv = cst.tile([P, 4 * S], FP8, tag="xv")
            wq = cst.tile([P, 1024], FP8, tag="wq")
            wk = cst.tile([P, 1024], FP8, tag="wk")
            wv = cst.tile([P, 1024], FP8, tag="wv")
            wo = cst.tile([P, 1024], BF16, tag="wo")
            bq = cst.tile([P, 2], F32, tag="bq")
            mk = cst.tile([P, P], BF16, tag="mk")

            # critical-path loads serialized on the sync queue at full
            # bandwidth (wq -> xq half0 -> wk -> xk half0 gate the first
            # exp); V-path loads are deferred via a head-0 filler so they
            # don't steal DMA bandwidth from the startup path
            nc.sync.dma_start(wq[:], wq_d[:])
            nc.sync.dma_start(bq[:], bq_d[:])
            nc.sync.dma_start(xq[:, 0:4096], xq_d[:, 0:4096])
            nc.sync.dma_start(wk[:], wk_d[:])
            nc.sync.dma_start(xk[:, 0:4096], xk_d[:, 0:4096])
            nc.sync.dma_start(mk[:], mk_d[:])
            nc.sync.dma_start(xq[:, 4096:8192], xq_d[:, 4096:8192])
            nc.sync.dma_start(xk[:, 4096:8192], xk_d[:, 4096:8192])
            nc.sync.dma_start(wo[:], wo_d[:])

            def load_v_inputs():
                nc.scalar.dma_start(wv[:], wv_d[:])
                nc.scalar.dma_start(xv[:, 0:4096], xv_d[:, 0:4096])
                nc.scalar.dma_start(xv[:, 4096:8192], xv_d[:, 4096:8192])

            QT = [act.tile([P, S], BF16, tag=f"QT{p}", name=f"QT{p}") for p in range(NPAIR)]
            KT = [act.tile([P, S], BF16, tag=f"KT{p}", name=f"KT{p}") for p in range(NPAIR)]
            Vn = [act.tile([P, 16 * 192], FP8, tag=f"Vn{p}", name=f"Vn{p}") for p in range(NPAIR)]
            XT = [act.tile([P, S], BF16, tag=f"XT{p}", name=f"XT{p}") for p in range(NPAIR)]

            def gview(t, g):
                # inputs are g-major: block (g, ch) of 1024 cols at (4g+ch)*1024
                return t[:, 4096 * g:4096 * (g + 1)].rearrange(
                    "p (ch c) -> p ch c", ch=4)

            def proj_qk_piece(p, is_q, g, act_evict=False):
                # one 1024-col psum group of the Q or K projection (fp8 DR)
                src4 = gview(xq if is_q else xk, g)
                wt = wq if is_q else wk
                dstT = QT[p] if is_q else KT[p]
                ps = stg.tile([P, 1024], F32, tag="stg")
                for half in range(2):
                    q0 = 512 * half
                    for cp in range(2):
                        wview = wt[:, (p * 2 + cp) * 256:(p * 2 + cp + 1) * 256]
                        nc.tensor.matmul(
                            ps[:, 512 * half:512 * (half + 1)],
                            wview.rearrange("p (two c) -> p two c", two=2),
                            src4[:, 2 * cp:2 * cp + 2, q0:q0 + 512],
                            start=(cp == 0), stop=(cp == 1),
                            perf_mode=DR)
                if is_q:
                    nc.vector.tensor_scalar(
                        dstT[:, 1024 * g:1024 * (g + 1)], ps[:],
                        0.125, bq[:, p:p + 1], MULT, ADD)
                elif act_evict:
                    nc.scalar.copy(dstT[:, 1024 * g:1024 * (g + 1)], ps[:])
                else:
                    nc.vector.tensor_copy(
                        dstT[:, 1024 * g:1024 * (g + 1)], ps[:])

            def proj_v_piece(g):
                # one psum tile (8 k-blocks) of the natural-layout V projection
                # one matmul per (k-block, chunk-pair) covers all 4 heads
                # (256-wide moving): psum tile holds 4 k-blocks
                ps = stg.tile([P, 1024], F32, tag="stg")
                xvg = gview(xv, g // 2)
                for kb in range(4):
                    kbg = 4 * g + kb          # global k-block 0..15
                    kbl = kbg - 8 * (g // 2)  # block within its g-half
                    for cp in range(2):
                        wview = wv[:, cp * 512:(cp + 1) * 512]
                        nc.tensor.matmul(
                            ps[:, 256 * kb:256 * (kb + 1)],
                            xvg[:, 2 * cp:2 * cp + 2, P * kbl:P * (kbl + 1)],
                            wview.rearrange("p (two c) -> p two c", two=2),
                            # start clears per PSUM bank: kb 0-1 bank A, 2-3 bank B
                            start=(cp == 0 and kb in (0, 2)),
                            stop=(cp == 1 and kb in (1, 3)),
                            perf_mode=DR, skip_group_check=True)
                # evict into the [V_h0|ones|V_h1] per-chunk pattern (both pairs)
                src = ps[:].rearrange("p (kb h c) -> p kb h c", kb=4, c=64)
                for pr in range(2):
                    dstp = Vn[pr][:].rearrange("p (kc t c) -> p kc t c",
                                               kc=16, c=64)
                    for hh in range(2):
                        nc.vector.tensor_copy(
                            dstp[:, 4 * g:4 * g + 4, 2 * hh:2 * hh + 1, :],
                            src[:, :, 2 * pr + hh:2 * pr + hh + 1, :])

            def emit_scores(h, fillers=(), tiles=None):
                """Scores + exp + masks for head h.  Filler pieces (work
                whose deps are already long satisfied) are emitted one per
                psum chunk so the PE queue carries independent work through
                the exp-pipeline stalls."""
                p, hh = h // 2, h % 2
                hr = slice(64 * hh, 64 * hh + 64)
                fillers = list(fillers)
                if tiles is None:
                    tiles = [None] * NJ

                def pop_filler():
                    while fillers:
                        fn = fillers.pop(0)
                        if fn is not None:
                            fn()
                        return

                for j in range(NJ):
                    w = W_J[j]
                    pt = pts.tile([P, 2 * w], FP8, tag=f"pt{j}")
                    tiles[j] = pt
                    for kc, sbase, slen in ((2 * j, 0, w - 128), (2 * j + 1, w, w)):
                        for c0, c1 in _chunks(0, slen, 1024):
                            ps = stg.tile([P, 1024], F32, tag="stg")
                            for a0, a1 in _chunks(c0, c1, 512):
                                nc.tensor.matmul(
                                    ps[:, a0 - c0:a1 - c0],
                                    KT[p][hr, P * kc:P * (kc + 1)],
                                    QT[p][hr, a0:a1],
                                    start=True, stop=True)
                            nc.scalar.activation(
                                pt[:, sbase + c0:sbase + c1],
                                ps[:, 0:c1 - c0], EXP)
                            pop_filler()
                    nc.vector.memset(pt[:, w - 128:w], 0.0)
                    nc.vector.tensor_tensor(
                        pt[:, w - 256:w - 128], pt[:, w - 256:w - 128], mk[:], MULT)
                    nc.vector.tensor_tensor(
                        pt[:, 2 * w - 128:2 * w], pt[:, 2 * w - 128:2 * w], mk[:], MULT)
                for fn in fillers:
                    if fn is not None:
                        fn()
                return tiles

            def av_pieces(h, tiles):
                """AV DR matmuls + norms for head h as filler callables for
                the next head's scores stream.  b-chunks {0,1} accumulate
                over all strip pairs first, then {2,3} re-read pt tiles of
                pairs 4-7 — only two PSUM banks live at a time."""
                p, hh = h // 2, h % 2
                hr = slice(64 * hh, 64 * hh + 64)
                orow, drow = (0, 64) if hh == 0 else (64, 0)
                avbs = [None] * 4

                def av(j, bs):
                    w = W_J[j]
                    pt2 = tiles[j][:].rearrange("p (two w) -> p two w", two=2)
                    vv = Vn[p][:].rearrange("p (kc c) -> p kc c", c=192)
                    for b in bs:
                        if 2 * b > j:
                            continue
                        if avbs[b] is None:
                            avbs[b] = avp.tile([P, 512], F32, tag="av",
                                               name=f"avb{h}_{b}")
                        ln = min(w, 512 * (b + 1)) - 512 * b
                        nc.tensor.matmul(
                            avbs[b][:, 0:ln],
                            vv[:, 2 * j:2 * j + 2, 64 * hh:64 * hh + 128],
                            pt2[:, :, 512 * b:512 * b + ln],
                            start=(j == 2 * b), stop=(j == NJ - 1),
                            perf_mode=DR, skip_group_check=True)

                def norm(bs):
                    for b in bs:
                        rec = rcp.tile([64, 1024], F32, tag="rec")
                        nc.vector.tensor_copy(rec[:, 0:512],
                                              avbs[b][drow:drow + 64, :])
                        nc.vector.reciprocal_approx_fast(rec[:, 512:1024],
                                                         rec[:, 0:512])
                        nc.vector.tensor_tensor(
                            XT[p][hr, 512 * b:512 * (b + 1)],
                            avbs[b][orow:orow + 64, :], rec[:, 512:1024], MULT)

                return ([lambda j=j: av(j, (0, 1)) for j in range(NJ)]
                        + [lambda: norm((0, 1))]
                        + [lambda j=j: av(j, (2, 3)) for j in range(4, NJ)]
                        + [lambda: norm((2, 3))])

            # pair-0 Q/K projections up front; everything else fills
            # scores-stream stalls
            proj_qk_piece(0, True, 0)
            proj_qk_piece(0, False, 0, act_evict=True)
            nc.gpsimd.memset(Vn[0][:], 1.0)
            nc.gpsimd.memset(Vn[1][:], 1.0)

            t0 = emit_scores(0, fillers=[
                   load_v_inputs, None, None,
                   lambda: proj_qk_piece(0, True, 1),
                   lambda: proj_qk_piece(0, False, 1)]
                 + [lambda g=g: proj_qk_piece(1, True, g) for g in range(2)]
                 + [lambda g=g: proj_qk_piece(1, False, g) for g in range(2)]
                 + [lambda g=g: proj_v_piece(g) for g in range(4)])
            a0 = av_pieces(0, t0)
            t1 = emit_scores(1, fillers=a0)
            a1 = av_pieces(1, t1)
            t2 = emit_scores(2, fillers=a1)
            a2 = av_pieces(2, t2)
            # head 3: interleave its own early AV pieces (2-pair exp lag)
            # after the previous head's full set, preserving avp ring order
            t3 = [None] * NJ
            a3 = av_pieces(3, t3)
            emit_scores(3, fillers=list(a2) + a3[0:5] + [None] + [a3[5]],
                        tiles=t3)
            for fn in a3[6:9]:    # av(6,b01), av(7,b01), norm01
                fn()
            b23 = list(a3[9:])    # av(4..7, b23), norm23

            # output projection: natural [q, d] bf16 partial result,
            # 2 q-tiles per psum group; the q<1024 half starts right after
            # norm01(h3), overlapping the b23 AV pieces
            def outproj_group(i2):
                ps = stg.tile([P, 1024], F32, tag="stg")
                for half in range(2):
                    i = 2 * i2 + half
                    for p in range(NPAIR):
                        nc.tensor.matmul(
                            ps[:, 512 * half:512 * (half + 1)],
                            XT[p][:, P * i:P * (i + 1)],
                            wo[:, 512 * p:512 * (p + 1)],
                            start=(p == 0), stop=(p == NPAIR - 1))
                ob = ost.tile([P, 1024], BF16, tag="ob")
                if i2 % 2 == 0:
                    nc.vector.tensor_copy(ob[:], ps[:])
                    nc.sync.dma_start(out_d[P * 2 * i2:P * (2 * i2 + 1), :],
                                      ob[:, 0:512])
                    nc.sync.dma_start(out_d[P * (2 * i2 + 1):P * (2 * i2 + 2), :],
                                      ob[:, 512:1024])
                else:
                    nc.scalar.copy(ob[:], ps[:])
                    nc.scalar.dma_start(out_d[P * 2 * i2:P * (2 * i2 + 1), :],
                                        ob[:, 0:512])
                    nc.scalar.dma_start(out_d[P * (2 * i2 + 1):P * (2 * i2 + 2), :],
                                        ob[:, 512:1024])

            for i2 in range(4):
                outproj_group(i2)
                if b23:
                    b23.pop(0)()
            for fn in b23:
                fn()
            for i2 in range(4, 8):
                outproj_group(i2)

    nc.compile()
    _cache["nc"] = nc
    return nc


def _host_prep(query, key, value, Wq, bq, Wk, bk, Wv, bv, Wo, bo):
    """Build the 8 per-core input maps."""
    def chunked_T(x):
        # [S, D] -> [128, 4*S], g-major: block (g, ch) at (4g+ch)*1024 holds
        # rows 128ch of x.T, cols [1024g : 1024(g+1))
        xT = np.ascontiguousarray(x.T)  # [512, S]
        return (xT.reshape(4, P, 2, 1024).transpose(1, 2, 0, 3)
                .reshape(P, 4 * S))

    xt = {b: {"q": chunked_T(query[b]).astype(F8),
              "k": chunked_T(key[b]).astype(F8),
              "v": chunked_T(value[b]).astype(F8)} for b in range(B)}

    kl = np.arange(P)[:, None]
    ql = np.arange(P)[None, :]
    mask = (kl > ql).astype(BF)

    in_maps = []
    for c in range(8):
        b, hp = c // 2, c % 2
        heads = range(4 * hp, 4 * hp + 4)

        def stat_pack(W):
            # stationary DR layout: block (p, cp, t) = W2[128*(2cp+t)] rows
            blocks = []
            for p in range(NPAIR):
                hg = 4 * hp + 2 * p
                W2 = np.concatenate([W[hg], W[hg + 1]], axis=1)  # [512, 128]
                for cp in range(2):
                    for t in range(2):
                        blocks.append(W2[P * (2 * cp + t):P * (2 * cp + t + 1), :])
            return np.concatenate(blocks, axis=1).astype(F8)  # [128, 1024]

        Wcat = np.concatenate([Wv[h] for h in heads], axis=1)  # [512, 256]
        wv_h = np.concatenate(
            [Wcat[P * i:P * (i + 1), :] for i in range(4)], axis=1).astype(F8)

        bq_h = np.stack(
            [np.concatenate([bq[4 * hp + 2 * p], bq[4 * hp + 2 * p + 1]])
             for p in range(NPAIR)], axis=1).astype(np.float32) / 8.0

        wo_h = np.concatenate(
            [Wo[64 * (4 * hp + 2 * p):64 * (4 * hp + 2 * p + 2), :]
             for p in range(NPAIR)], axis=1).astype(BF)  # [128, 1024]

        in_maps.append({
            "xq": xt[b]["q"], "xk": xt[b]["k"], "xv": xt[b]["v"],
            "wq": stat_pack(Wq), "wk": stat_pack(Wk), "wv": wv_h,
            "wo": wo_h, "bq": bq_h, "mask": mask,
        })
    return in_maps


def kernel(query, key, value, Wq, bq, Wk, bk, Wv, bv, Wo, bo):
    from concourse.bass_utils import run_bass_kernel_spmd

    args = [np.asarray(a, dtype=np.float32) for a in
            (query, key, value, Wq, bq, Wk, bk, Wv, bv, Wo, bo)]
    query, key, value, Wq, bq, Wk, bk, Wv, bv, Wo, bo = args

    nc = _build()
    in_maps = _host_prep(*args)
    res = run_bass_kernel_spmd(nc, in_maps, list(range(8)))

    # host gather: sum the two head-half partials + bias constant
    const = (bo + bv.reshape(-1) @ Wo).astype(np.float32)  # bv via softmax-sums-to-1
    out = np.empty((B, S, D), np.float32)
    for b in range(B):
        out[b] = (res.results[2 * b]["out"].astype(np.float32)
                  + res.results[2 * b + 1]["out"].astype(np.float32) + const)

    # exact host recompute of the last FIX_ROWS rows (tiny fan-in + q=S-1)
    scale = 1.0 / np.sqrt(A)
    for b in range(B):
        vm = value[b].mean(0)
        x = np.concatenate([vm @ Wv[h] + bv[h] for h in range(H)])
        out[b, S - 1, :] = x @ Wo + bo
        for q in range(S - FIX_ROWS, S - 1):
            ks = np.arange(q + 1, S)
            xrow = []
            for h in range(H):
                qh = query[b, q] @ Wq[h] + bq[h]
                kh = key[b, ks] @ Wk[h] + bk[h]
                vh = value[b, ks] @ Wv[h] + bv[h]
                sc = (kh @ qh) * scale
                sc -= sc.max()
                pw = np.exp(sc)
                pw /= pw.sum()
                xrow.append(pw @ vh)
            out[b, q, :] = np.concatenate(xrow) @ Wo + bo
    return out


# revision 36
# speedup vs baseline: 1.0222x; 1.0222x over previous
"""Multi-head attention (strictly-future mask) on 8 TRN2 cores — v2.

Reference math (B=4, S=2048, D=512, H=8, A=64):
    q/k/v = per-head projections                              [B,H,S,A]
    scores = q @ k^T / 8, lower triangle (k <= q) masked to -1e9
    out = concat_heads(softmax(scores) @ v) @ Wo + bo         [B,S,D]

Sharding: head-parallel within a batch — core c = (batch b = c//2,
head-half hp = c%2).  Each core computes 4 heads (= 2 stacked head
pairs) over the FULL 2048-query range, producing a partial output
summed on the host (Wo split along its input axis, per the TP hint);
host reduction replaces the all-reduce.

Bias algebra: softmax cancels any per-query additive score term, so
bk is dropped (its q-dependent term cancels) and only bq is kept on Q
(it produces the surviving per-key term).  bv contributes exactly
bv @ Wo_head (softmax weights sum to 1) — folded with bo into a host
constant.  K/V psum evictions are therefore pure copies.

Device dataflow (per core):
  * Q/K projections (transposed [a,S] layout) and V projection
    (natural [k,a] layout, input as stationary operand — no PE
    transposes needed) run as fp8e4 DoubleRow matmuls: contraction
    512 = 2x(2x128) chunk pairs at 0.5 cycles/column.
  * Scores are computed transposed (S^T[k,q]) in bf16; per head-pair
    strips kc carry a uniform pair width w_j = 256(j+1) so DoubleRow
    can pair even/odd k-strips in the AV stage.  exp() on the scalar
    engine writes P^T directly as fp8e4; diagonal blocks are masked
    post-exp by a 0/1 tril multiply; the even strip's overhang block
    is never computed — just zero-filled.
  * AV runs as fp8 DoubleRow over strip pairs with the [V|ones|V]
    stationary trick replicating softmax denominators; normalization
    divides via copy + reciprocal_approx_fast on DVE.
  * Output projection consumes X^T per 128-query tile, producing the
    natural [2048, 512] f32 partial output.

The last 32 query rows (tiny attention fan-in, where fp8 quantization
noise is largest — and q = S-1 is 0/0) are recomputed exactly on the
host during the gather.
"""

import numpy as np
import ml_dtypes

B, S, D, H, A = 4, 2048, 512, 8, 64
P = 128
NPAIR = 2            # head pairs per core
NHEAD = 4            # heads per core
NJ = 8               # strip pairs per head
FIX_ROWS = 64        # host-recomputed tail rows
BF = ml_dtypes.bfloat16
F8 = ml_dtypes.float8_e4m3

W_J = [256 * (j + 1) for j in range(NJ)]   # uniform pair strip widths

_cache = {}


def _chunks(lo, hi, step):
    out = []
    while lo < hi:
        out.append((lo, min(hi, lo + step)))
        lo = out[-1][1]
    return out


def _build():
    if "nc" in _cache:
        return _cache["nc"]

    import concourse.bacc as bacc
    import concourse.mybir as mybir
    import concourse.tile as tile

    F32 = mybir.dt.float32
    BF16 = mybir.dt.bfloat16
    FP8 = mybir.dt.float8e4
    MULT = mybir.AluOpType.mult
    ADD = mybir.AluOpType.add
    EXP = mybir.ActivationFunctionType.Exp
    DR = mybir.MatmulPerfMode.DoubleRow

    nc = bacc.Bacc("TRN2", target_bir_lowering=False, debug=False, num_devices=8)

    xq_d = nc.dram_tensor("xq", [P, 4 * S], FP8, kind="ExternalInput")
    xk_d = nc.dram_tensor("xk", [P, 4 * S], FP8, kind="ExternalInput")
    xv_d = nc.dram_tensor("xv", [P, 4 * S], FP8, kind="ExternalInput")
    wq_d = nc.dram_tensor("wq", [P, 1024], FP8, kind="ExternalInput")
    wk_d = nc.dram_tensor("wk", [P, 1024], FP8, kind="ExternalInput")
    wv_d = nc.dram_tensor("wv", [P, 1024], FP8, kind="ExternalInput")
    wo_d = nc.dram_tensor("wo", [P, 1024], BF16, kind="ExternalInput")
    bq_d = nc.dram_tensor("bq", [P, 2], F32, kind="ExternalInput")
    mk_d = nc.dram_tensor("mask", [P, P], BF16, kind="ExternalInput")
    out_d = nc.dram_tensor("out", [S, D], BF16, kind="ExternalOutput")

    with tile.TileContext(nc) as tc:
        with (
            tc.tile_pool(name="cst", bufs=1) as cst,
            tc.tile_pool(name="act", bufs=1) as act,
            tc.tile_pool(name="pts", bufs=3) as pts,
            tc.tile_pool(name="rcp", bufs=4) as rcp,
            tc.tile_pool(name="ost", bufs=3) as ost,
            tc.tile_pool(name="stg", bufs=3, space="PSUM") as stg,
            tc.tile_pool(name="avp", bufs=2, space="PSUM") as avp,
        ):
            xq = cst.tile([P, 4 * S], FP8, tag="xq")
            xk = cst.tile([P, 4 * S], FP8, tag="xk")
            xv = cst.tile([P, 4 * S], FP8, tag="xv")
            wq = cst.tile([P, 1024], FP8, tag="wq")
            wk = cst.tile([P, 1024], FP8, tag="wk")
            wv = cst.tile([P, 1024], FP8, tag="wv")
            wo = cst.tile([P, 1024], BF16, tag="wo")
            bq = cst.tile([P, 2], F32, tag="bq")
            mk = cst.tile([P, P], BF16, tag="mk")

            # loads split across the two HW DGE queues (sync=Q-path,
            # scalar=K/V-path); one DMA per tensor for full-row transfers
            nc.sync.dma_start(wq[:], wq_d[:])
            nc.sync.dma_start(bq[:], bq_d[:])
            nc.scalar.dma_start(wk[:], wk_d[:])
            nc.sync.dma_start(xq[:, 0:4096], xq_d[:, 0:4096])
            nc.scalar.dma_start(xk[:, 0:4096], xk_d[:, 0:4096])
            nc.sync.dma_start(xq[:, 4096:8192], xq_d[:, 4096:8192])
            nc.scalar.dma_start(xk[:, 4096:8192], xk_d[:, 4096:8192])
            nc.sync.dma_start(mk[:], mk_d[:])
            nc.scalar.dma_start(wv[:], wv_d[:])
            nc.scalar.dma_start(xv[:, 0:4096], xv_d[:, 0:4096])
            nc.scalar.dma_start(xv[:, 4096:8192], xv_d[:, 4096:8192])
            nc.sync.dma_start(wo[:], wo_d[:])

            QT = [act.tile([P, S], BF16, tag=f"QT{p}", name=f"QT{p}") for p in range(NPAIR)]
            KT = [act.tile([P, S], BF16, tag=f"KT{p}", name=f"KT{p}") for p in range(NPAIR)]
            Vn = [act.tile([P, 16 * 192], FP8, tag=f"Vn{p}", name=f"Vn{p}") for p in range(NPAIR)]
            XT = [act.tile([P, S], BF16, tag=f"XT{p}", name=f"XT{p}") for p in range(NPAIR)]

            def gview(t, g):
                # inputs are g-major: block (g, ch) of 1024 cols at (4g+ch)*1024
                return t[:, 4096 * g:4096 * (g + 1)].rearrange(
                    "p (ch c) -> p ch c", ch=4)

            def proj_qk_piece(p, is_q, g, act_evict=False):
                # one 1024-col psum group of the Q or K projection (fp8 DR)
                src4 = gview(xq if is_q else xk, g)
                wt = wq if is_q else wk
                dstT = QT[p] if is_q else KT[p]
                ps = stg.tile([P, 1024], F32, tag="stg")
                for half in range(2):
                    q0 = 512 * half
                    for cp in range(2):
                        wview = wt[:, (p * 2 + cp) * 256:(p * 2 + cp + 1) * 256]
                        nc.tensor.matmul(
                            ps[:, 512 * half:512 * (half + 1)],
                            wview.rearrange("p (two c) -> p two c", two=2),
                            src4[:, 2 * cp:2 * cp + 2, q0:q0 + 512],
                            start=(cp == 0), stop=(cp == 1),
                            perf_mode=DR)
                if is_q:
                    nc.vector.tensor_scalar(
                        dstT[:, 1024 * g:1024 * (g + 1)], ps[:],
                        0.125, bq[:, p:p + 1], MULT, ADD)
                elif act_evict:
                    nc.scalar.copy(dstT[:, 1024 * g:1024 * (g + 1)], ps[:])
                else:
                    nc.vector.tensor_copy(
                        dstT[:, 1024 * g:1024 * (g + 1)], ps[:])

            def proj_v_piece(g):
                # one psum tile (8 k-blocks) of the natural-layout V projection
                # one matmul per (k-block, chunk-pair) covers all 4 heads
                # (256-wide moving): psum tile holds 4 k-blocks
                ps = stg.tile([P, 1024], F32, tag="stg")
                xvg = gview(xv, g // 2)
                for kb in range(4):
                    kbg = 4 * g + kb          # global k-block 0..15
                    kbl = kbg - 8 * (g // 2)  # block within its g-half
                    for cp in range(2):
                        wview = wv[:, cp * 512:(cp + 1) * 512]
                        nc.tensor.matmul(
                            ps[:, 256 * kb:256 * (kb + 1)],
                            xvg[:, 2 * cp:2 * cp + 2, P * kbl:P * (kbl + 1)],
                            wview.rearrange("p (two c) -> p two c", two=2),
                            # start clears per PSUM bank: kb 0-1 bank A, 2-3 bank B
                            start=(cp == 0 and kb in (0, 2)),
                            stop=(cp == 1 and kb in (1, 3)),
                            perf_mode=DR, skip_group_check=True)
                # evict into the [V_h0|ones|V_h1] per-chunk pattern (both pairs)
                src = ps[:].rearrange("p (kb h c) -> p kb h c", kb=4, c=64)
                for pr in range(2):
                    dstp = Vn[pr][:].rearrange("p (kc t c) -> p kc t c",
                                               kc=16, c=64)
                    for hh in range(2):
                        nc.vector.tensor_copy(
                            dstp[:, 4 * g:4 * g + 4, 2 * hh:2 * hh + 1, :],
                            src[:, :, 2 * pr + hh:2 * pr + hh + 1, :])

            def emit_scores(h, fillers=(), tiles=None):
                """Scores + exp + masks for head h.  Filler pieces (work
                whose deps are already long satisfied) are emitted one per
                psum chunk so the PE queue carries independent work through
                the exp-pipeline stalls."""
                p, hh = h // 2, h % 2
                hr = slice(64 * hh, 64 * hh + 64)
                fillers = list(fillers)
                if tiles is None:
                    tiles = [None] * NJ

                def pop_filler():
                    while fillers:
                        fn = fillers.pop(0)
                        if fn is not None:
                            fn()
                        return

                for j in range(NJ):
                    w = W_J[j]
                    pt = pts.tile([P, 2 * w], FP8, tag=f"pt{j}")
                    tiles[j] = pt
                    for kc, sbase, slen in ((2 * j, 0, w - 128), (2 * j + 1, w, w)):
                        for c0, c1 in _chunks(0, slen, 1024):
                            ps = stg.tile([P, 1024], F32, tag="stg")
                            for a0, a1 in _chunks(c0, c1, 512):
                                nc.tensor.matmul(
                                    ps[:, a0 - c0:a1 - c0],
                                    KT[p][hr, P * kc:P * (kc + 1)],
                                    QT[p][hr, a0:a1],
                                    start=True, stop=True)
                            nc.scalar.activation(
                                pt[:, sbase + c0:sbase + c1],
                                ps[:, 0:c1 - c0], EXP)
                            pop_filler()
                    nc.vector.memset(pt[:, w - 128:w], 0.0)
                    nc.vector.tensor_tensor(
                        pt[:, w - 256:w - 128], pt[:, w - 256:w - 128], mk[:], MULT)
                    nc.vector.tensor_tensor(
                        pt[:, 2 * w - 128:2 * w], pt[:, 2 * w - 128:2 * w], mk[:], MULT)
                for fn in fillers:
                    if fn is not None:
                        fn()
                return tiles

            def av_pieces(h, tiles):
                """AV DR matmuls + norms for head h as filler callables for
                the next head's scores stream.  b-chunks {0,1} accumulate
                over all strip pairs first, then {2,3} re-read pt tiles of
                pairs 4-7 — only two PSUM banks live at a time."""
                p, hh = h // 2, h % 2
                hr = slice(64 * hh, 64 * hh + 64)
                orow, drow = (0, 64) if hh == 0 else (64, 0)
                avbs = [None] * 4

                def av(j, bs):
                    w = W_J[j]
                    pt2 = tiles[j][:].rearrange("p (two w) -> p two w", two=2)
                    vv = Vn[p][:].rearrange("p (kc c) -> p kc c", c=192)
                    for b in bs:
                        if 2 * b > j:
                            continue
                        if avbs[b] is None:
                            avbs[b] = avp.tile([P, 512], F32, tag="av",
                                               name=f"avb{h}_{b}")
                        ln = min(w, 512 * (b + 1)) - 512 * b
                        nc.tensor.matmul(
                            avbs[b][:, 0:ln],
                            vv[:, 2 * j:2 * j + 2, 64 * hh:64 * hh + 128],
                            pt2[:, :, 512 * b:512 * b + ln],
                            start=(j == 2 * b), stop=(j == NJ - 1),
                            perf_mode=DR, skip_group_check=True)

                def norm(bs):
                    for b in bs:
                        rec = rcp.tile([64, 1024], F32, tag="rec")
                        nc.vector.tensor_copy(rec[:, 0:512],
                                              avbs[b][drow:drow + 64, :])
                        nc.vector.reciprocal_approx_fast(rec[:, 512:1024],
                                                         rec[:, 0:512])
                        nc.vector.tensor_tensor(
                            XT[p][hr, 512 * b:512 * (b + 1)],
                            avbs[b][orow:orow + 64, :], rec[:, 512:1024], MULT)

                return ([lambda j=j: av(j, (0, 1)) for j in range(NJ)]
                        + [lambda: norm((0, 1))]
                        + [lambda j=j: av(j, (2, 3)) for j in range(4, NJ)]
                        + [lambda: norm((2, 3))])

            # pair-0 Q/K projections up front; everything else fills
            # scores-stream stalls
            proj_qk_piece(0, True, 0)
            proj_qk_piece(0, False, 0, act_evict=True)
            nc.gpsimd.memset(Vn[0][:], 1.0)
            nc.gpsimd.memset(Vn[1][:], 1.0)

            t0 = emit_scores(0, fillers=[
                   lambda: proj_qk_piece(0, True, 1),
                   lambda: proj_qk_piece(0, False, 1)]
                 + [lambda g=g: proj_qk_piece(1, True, g) for g in range(2)]
                 + [lambda g=g: proj_qk_piece(1, False, g) for g in range(2)]
                 + [lambda g=g: proj_v_piece(g) for g in range(4)])
            a0 = av_pieces(0, t0)
            t1 = emit_scores(1, fillers=a0)
            a1 = av_pieces(1, t1)
            t2 = emit_scores(2, fillers=a1)
            a2 = av_pieces(2, t2)
            # head 3: interleave its own early AV pieces (2-pair exp lag)
            # after the previous head's full set, preserving avp ring order
            t3 = [None] * NJ
            a3 = av_pieces(3, t3)
            emit_scores(3, fillers=list(a2) + a3[0:5] + [None] + [a3[5]],
                        tiles=t3)
            for fn in a3[6:9]:    # av(6,b01), av(7,b01), norm01
                fn()
            b23 = list(a3[9:])    # av(4..7, b23), norm23

            # output projection: natural [q, d] bf16 partial result,
            # 2 q-tiles per psum group; the q<1024 half starts right after
            # norm01(h3), overlapping the b23 AV pieces
            def outproj_group(i2):
                ps = stg.tile([P, 1024], F32, tag="stg")
                for half in range(2):
                    i = 2 * i2 + half
                    for p in range(NPAIR):
                        nc.tensor.matmul(
                            ps[:, 512 * half:512 * (half + 1)],
                            XT[p][:, P * i:P * (i + 1)],
                            wo[:, 512 * p:512 * (p + 1)],
                            start=(p == 0), stop=(p == NPAIR - 1))
                ob = ost.tile([P, 1024], BF16, tag="ob")
                if i2 % 2 == 0:
                    nc.vector.tensor_copy(ob[:], ps[:])
                    nc.sync.dma_start(out_d[P * 2 * i2:P * (2 * i2 + 1), :],
                                      ob[:, 0:512])
                    nc.sync.dma_start(out_d[P * (2 * i2 + 1):P * (2 * i2 + 2), :],
                                      ob[:, 512:1024])
                else:
                    nc.scalar.copy(ob[:], ps[:])
                    nc.scalar.dma_start(out_d[P * 2 * i2:P * (2 * i2 + 1), :],
                                        ob[:, 0:512])
                    nc.scalar.dma_start(out_d[P * (2 * i2 + 1):P * (2 * i2 + 2), :],
                                        ob[:, 512:1024])

            for i2 in range(4):
                outproj_group(i2)
                if b23:
                    b23.pop(0)()
            for fn in b23:
                fn()
            for i2 in range(4, 8):
                outproj_group(i2)

    nc.compile()
    _cache["nc"] = nc
    return nc


def _host_prep(query, key, value, Wq, bq, Wk, bk, Wv, bv, Wo, bo):
    """Build the 8 per-core input maps."""
    def chunked_T(x):
        # [S, D] -> [128, 4*S], g-major: block (g, ch) at (4g+ch)*1024 holds
        # rows 128ch of x.T, cols [1024g : 1024(g+1))
        xT = np.ascontiguousarray(x.T)  # [512, S]
        return (xT.reshape(4, P, 2, 1024).transpose(1, 2, 0, 3)
                .reshape(P, 4 * S))

    xt = {b: {"q": chunked_T(query[b]).astype(F8),
              "k": chunked_T(key[b]).astype(F8),
              "v": chunked_T(value[b]).astype(F8)} for b in range(B)}

    kl = np.arange(P)[:, None]
    ql = np.arange(P)[None, :]
    mask = (kl > ql).astype(BF)

    in_maps = []
    for c in range(8):
        b, hp = c // 2, c % 2
        heads = range(4 * hp, 4 * hp + 4)

        def stat_pack(W):
            # stationary DR layout: block (p, cp, t) = W2[128*(2cp+t)] rows
            blocks = []
            for p in range(NPAIR):
                hg = 4 * hp + 2 * p
                W2 = np.concatenate([W[hg], W[hg + 1]], axis=1)  # [512, 128]
                for cp in range(2):
                    for t in range(2):
                        blocks.append(W2[P * (2 * cp + t):P * (2 * cp + t + 1), :])
            return np.concatenate(blocks, axis=1).astype(F8)  # [128, 1024]

        Wcat = np.concatenate([Wv[h] for h in heads], axis=1)  # [512, 256]
        wv_h = np.concatenate(
            [Wcat[P * i:P * (i + 1), :] for i in range(4)], axis=1).astype(F8)

        bq_h = np.stack(
            [np.concatenate([bq[4 * hp + 2 * p], bq[4 * hp + 2 * p + 1]])
             for p in range(NPAIR)], axis=1).astype(np.float32) / 8.0

        wo_h = np.concatenate(
            [Wo[64 * (4 * hp + 2 * p):64 * (4 * hp + 2 * p + 2), :]
             for p in range(NPAIR)], axis=1).astype(BF)  # [128, 1024]

        in_maps.append({
            "xq": xt[b]["q"], "xk": xt[b]["k"], "xv": xt[b]["v"],
            "wq": stat_pack(Wq), "wk": stat_pack(Wk), "wv": wv_h,
            "wo": wo_h, "bq": bq_h, "mask": mask,
        })
    return in_maps


def kernel(query, key, value, Wq, bq, Wk, bk, Wv, bv, Wo, bo):
    from concourse.bass_utils import run_bass_kernel_spmd

    args = [np.asarray(a, dtype=np.float32) for a in
            (query, key, value, Wq, bq, Wk, bk, Wv, bv, Wo, bo)]
    query, key, value, Wq, bq, Wk, bk, Wv, bv, Wo, bo = args

    nc = _build()
    in_maps = _host_prep(*args)
    res = run_bass_kernel_spmd(nc, in_maps, list(range(8)))

    # host gather: sum the two head-half partials + bias constant
    const = (bo + bv.reshape(-1) @ Wo).astype(np.float32)  # bv via softmax-sums-to-1
    out = np.empty((B, S, D), np.float32)
    for b in range(B):
        out[b] = (res.results[2 * b]["out"].astype(np.float32)
                  + res.results[2 * b + 1]["out"].astype(np.float32) + const)

    # exact host recompute of the last FIX_ROWS rows (tiny fan-in + q=S-1)
    scale = 1.0 / np.sqrt(A)
    for b in range(B):
        vm = value[b].mean(0)
        x = np.concatenate([vm @ Wv[h] + bv[h] for h in range(H)])
        out[b, S - 1, :] = x @ Wo + bo
        for q in range(S - FIX_ROWS, S - 1):
            ks = np.arange(q + 1, S)
            xrow = []
            for h in range(H):
                qh = query[b, q] @ Wq[h] + bq[h]
                kh = key[b, ks] @ Wk[h] + bk[h]
                vh = value[b, ks] @ Wv[h] + bv[h]
                sc = (kh @ qh) * scale
                sc -= sc.max()
                pw = np.exp(sc)
                pw /= pw.sum()
                xrow.append(pw @ vh)
            out[b, q, :] = np.concatenate(xrow) @ Wo + bo
    return out


# revision 38
# speedup vs baseline: 1.0309x; 1.0086x over previous
"""Multi-head attention (strictly-future mask) on 8 TRN2 cores — v2.

Reference math (B=4, S=2048, D=512, H=8, A=64):
    q/k/v = per-head projections                              [B,H,S,A]
    scores = q @ k^T / 8, lower triangle (k <= q) masked to -1e9
    out = concat_heads(softmax(scores) @ v) @ Wo + bo         [B,S,D]

Sharding: head-parallel within a batch — core c = (batch b = c//2,
head-half hp = c%2).  Each core computes 4 heads (= 2 stacked head
pairs) over the FULL 2048-query range, producing a partial output
summed on the host (Wo split along its input axis, per the TP hint);
host reduction replaces the all-reduce.

Bias algebra: softmax cancels any per-query additive score term, so
bk is dropped (its q-dependent term cancels) and only bq is kept on Q
(it produces the surviving per-key term).  bv contributes exactly
bv @ Wo_head (softmax weights sum to 1) — folded with bo into a host
constant.  K/V psum evictions are therefore pure copies.

Device dataflow (per core):
  * Q/K projections (transposed [a,S] layout) and V projection
    (natural [k,a] layout, input as stationary operand — no PE
    transposes needed) run as fp8e4 DoubleRow matmuls: contraction
    512 = 2x(2x128) chunk pairs at 0.5 cycles/column.
  * Scores are computed transposed (S^T[k,q]) in bf16; per head-pair
    strips kc carry a uniform pair width w_j = 256(j+1) so DoubleRow
    can pair even/odd k-strips in the AV stage.  exp() on the scalar
    engine writes P^T directly as fp8e4; diagonal blocks are masked
    post-exp by a 0/1 tril multiply; the even strip's overhang block
    is never computed — just zero-filled.
  * AV runs as fp8 DoubleRow over strip pairs with the [V|ones|V]
    stationary trick replicating softmax denominators; normalization
    divides via copy + reciprocal_approx_fast on DVE.
  * Output projection consumes X^T per 128-query tile, producing the
    natural [2048, 512] f32 partial output.

Orchestration: all streams are interleaved at psum-chunk granularity —
each head's scores/exp stream carries "filler" pieces (the previous
head's AV matmuls + norms, pair-1 projections during head 0) so the
in-order PE queue always holds independent work behind any
exp-pipeline stall.  PSUM: 6 banks of depth-3 [128,1024] staging ring
+ 2 banks of AV accumulators (b-chunks {0,1} then {2,3}, re-reading
the persistent fp8 P^T tiles).  The output projection's first q-half
overlaps the final AV/norm work.

The last FIX_ROWS query rows (tiny attention fan-in, where fp8
quantization noise is largest — and q = S-1 is 0/0) are recomputed
exactly on the host during the gather.
"""

import numpy as np
import ml_dtypes

B, S, D, H, A = 4, 2048, 512, 8, 64
P = 128
NPAIR = 2            # head pairs per core
NHEAD = 4            # heads per core
NJ = 8               # strip pairs per head
FIX_ROWS = 64        # host-recomputed tail rows
BF = ml_dtypes.bfloat16
F8 = ml_dtypes.float8_e4m3

W_J = [256 * (j + 1) for j in range(NJ)]   # uniform pair strip widths

_cache = {}


def _chunks(lo, hi, step):
    out = []
    while lo < hi:
        out.append((lo, min(hi, lo + step)))
        lo = out[-1][1]
    return out


def _build():
    if "nc" in _cache:
        return _cache["nc"]

    import concourse.bacc as bacc
    import concourse.mybir as mybir
    import concourse.tile as tile

    F32 = mybir.dt.float32
    BF16 = mybir.dt.bfloat16
    FP8 = mybir.dt.float8e4
    MULT = mybir.AluOpType.mult
    ADD = mybir.AluOpType.add
    EXP = mybir.ActivationFunctionType.Exp
    DR = mybir.MatmulPerfMode.DoubleRow

    nc = bacc.Bacc("TRN2", target_bir_lowering=False, debug=False, num_devices=8)

    xq_d = nc.dram_tensor("xq", [P, 4 * S], FP8, kind="ExternalInput")
    xk_d = nc.dram_tensor("xk", [P, 4 * S], FP8, kind="ExternalInput")
    xv_d = nc.dram_tensor("xv", [P, 4 * S], FP8, kind="ExternalInput")
    wq_d = nc.dram_tensor("wq", [P, 1024], FP8, kind="ExternalInput")
    wk_d = nc.dram_tensor("wk", [P, 1024], FP8, kind="ExternalInput")
    wv_d = nc.dram_tensor("wv", [P, 1024], FP8, kind="ExternalInput")
    wo_d = nc.dram_tensor("wo", [P, 1024], BF16, kind="ExternalInput")
    bq_d = nc.dram_tensor("bq", [P, 2], F32, kind="ExternalInput")
    mk_d = nc.dram_tensor("mask", [P, P], BF16, kind="ExternalInput")
    out_d = nc.dram_tensor("out", [S, D], BF16, kind="ExternalOutput")

    with tile.TileContext(nc) as tc:
        with (
            tc.tile_pool(name="cst", bufs=1) as cst,
            tc.tile_pool(name="act", bufs=1) as act,
            tc.tile_pool(name="pts", bufs=3) as pts,
            tc.tile_pool(name="rcp", bufs=4) as rcp,
            tc.tile_pool(name="ost", bufs=3) as ost,
            tc.tile_pool(name="stg", bufs=3, space="PSUM") as stg,
            tc.tile_pool(name="avp", bufs=2, space="PSUM") as avp,
        ):
            xq = cst.tile([P, 4 * S], FP8, tag="xq")
            xk = cst.tile([P, 4 * S], FP8, tag="xk")
            xv = cst.tile([P, 4 * S], FP8, tag="xv")
            wq = cst.tile([P, 1024], FP8, tag="wq")
            wk = cst.tile([P, 1024], FP8, tag="wk")
            wv = cst.tile([P, 1024], FP8, tag="wv")
            wo = cst.tile([P, 1024], BF16, tag="wo")
            bq = cst.tile([P, 2], F32, tag="bq")
            mk = cst.tile([P, P], BF16, tag="mk")

            # loads split across the two HW DGE queues (sync=Q-path,
            # scalar=K/V-path); one DMA per tensor for full-row transfers
            nc.sync.dma_start(wq[:], wq_d[:])
            nc.sync.dma_start(bq[:], bq_d[:])
            nc.scalar.dma_start(wk[:], wk_d[:])
            nc.sync.dma_start(xq[:, 0:4096], xq_d[:, 0:4096])
            nc.scalar.dma_start(xk[:, 0:4096], xk_d[:, 0:4096])
            nc.sync.dma_start(xq[:, 4096:8192], xq_d[:, 4096:8192])
            nc.scalar.dma_start(xk[:, 4096:8192], xk_d[:, 4096:8192])
            nc.sync.dma_start(mk[:], mk_d[:])
            nc.scalar.dma_start(wv[:], wv_d[:])
            nc.scalar.dma_start(xv[:, 0:4096], xv_d[:, 0:4096])
            nc.scalar.dma_start(xv[:, 4096:8192], xv_d[:, 4096:8192])
            nc.sync.dma_start(wo[:], wo_d[:])

            QT = [act.tile([P, S], BF16, tag=f"QT{p}", name=f"QT{p}") for p in range(NPAIR)]
            KT = [act.tile([P, S], BF16, tag=f"KT{p}", name=f"KT{p}") for p in range(NPAIR)]
            Vn = [act.tile([P, 16 * 192], FP8, tag=f"Vn{p}", name=f"Vn{p}") for p in range(NPAIR)]
            XT = [act.tile([P, S], BF16, tag=f"XT{p}", name=f"XT{p}") for p in range(NPAIR)]

            def gview(t, g):
                # inputs are g-major: block (g, ch) of 1024 cols at (4g+ch)*1024
                return t[:, 4096 * g:4096 * (g + 1)].rearrange(
                    "p (ch c) -> p ch c", ch=4)

            def proj_qk_piece(p, is_q, g, act_evict=False):
                # one 1024-col psum group of the Q or K projection (fp8 DR)
                src4 = gview(xq if is_q else xk, g)
                wt = wq if is_q else wk
                dstT = QT[p] if is_q else KT[p]
                ps = stg.tile([P, 1024], F32, tag="stg")
                for half in range(2):
                    q0 = 512 * half
                    for cp in range(2):
                        wview = wt[:, (p * 2 + cp) * 256:(p * 2 + cp + 1) * 256]
                        nc.tensor.matmul(
                            ps[:, 512 * half:512 * (half + 1)],
                            wview.rearrange("p (two c) -> p two c", two=2),
                            src4[:, 2 * cp:2 * cp + 2, q0:q0 + 512],
                            start=(cp == 0), stop=(cp == 1),
                            perf_mode=DR)
                if is_q:
                    nc.vector.tensor_scalar(
                        dstT[:, 1024 * g:1024 * (g + 1)], ps[:],
                        0.125, bq[:, p:p + 1], MULT, ADD)
                elif act_evict:
                    nc.scalar.copy(dstT[:, 1024 * g:1024 * (g + 1)], ps[:])
                else:
                    nc.vector.tensor_copy(
                        dstT[:, 1024 * g:1024 * (g + 1)], ps[:])

            def proj_v_piece(g):
                # one psum tile (8 k-blocks) of the natural-layout V projection
                # one matmul per (k-block, chunk-pair) covers all 4 heads
                # (256-wide moving): psum tile holds 4 k-blocks
                ps = stg.tile([P, 1024], F32, tag="stg")
                xvg = gview(xv, g // 2)
                for kb in range(4):
                    kbg = 4 * g + kb          # global k-block 0..15
                    kbl = kbg - 8 * (g // 2)  # block within its g-half
                    for cp in range(2):
                        wview = wv[:, cp * 512:(cp + 1) * 512]
                        nc.tensor.matmul(
                            ps[:, 256 * kb:256 * (kb + 1)],
                            xvg[:, 2 * cp:2 * cp + 2, P * kbl:P * (kbl + 1)],
                            wview.rearrange("p (two c) -> p two c", two=2),
                            # start clears per PSUM bank: kb 0-1 bank A, 2-3 bank B
                            start=(cp == 0 and kb in (0, 2)),
                            stop=(cp == 1 and kb in (1, 3)),
                            perf_mode=DR, skip_group_check=True)
                # evict into the [V_h0|ones|V_h1] per-chunk pattern (both pairs)
                src = ps[:].rearrange("p (kb h c) -> p kb h c", kb=4, c=64)
                for pr in range(2):
                    dstp = Vn[pr][:].rearrange("p (kc t c) -> p kc t c",
                                               kc=16, c=64)
                    for hh in range(2):
                        nc.vector.tensor_copy(
                            dstp[:, 4 * g:4 * g + 4, 2 * hh:2 * hh + 1, :],
                            src[:, :, 2 * pr + hh:2 * pr + hh + 1, :])

            def emit_scores(h, fillers=(), tiles=None):
                """Scores + exp + masks for head h.  Filler pieces (work
                whose deps are already long satisfied) are emitted one per
                psum chunk so the PE queue carries independent work through
                the exp-pipeline stalls."""
                p, hh = h // 2, h % 2
                hr = slice(64 * hh, 64 * hh + 64)
                fillers = list(fillers)
                if tiles is None:
                    tiles = [None] * NJ

                def pop_filler():
                    while fillers:
                        fn = fillers.pop(0)
                        if fn is not None:
                            fn()
                        return

                for j in range(NJ):
                    w = W_J[j]
                    pt = pts.tile([P, 2 * w], FP8, tag=f"pt{j}")
                    tiles[j] = pt
                    for kc, sbase, slen in ((2 * j, 0, w - 128), (2 * j + 1, w, w)):
                        for c0, c1 in _chunks(0, slen, 1024):
                            ps = stg.tile([P, 1024], F32, tag="stg")
                            for a0, a1 in _chunks(c0, c1, 512):
                                nc.tensor.matmul(
                                    ps[:, a0 - c0:a1 - c0],
                                    KT[p][hr, P * kc:P * (kc + 1)],
                                    QT[p][hr, a0:a1],
                                    start=True, stop=True)
                            nc.scalar.activation(
                                pt[:, sbase + c0:sbase + c1],
                                ps[:, 0:c1 - c0], EXP)
                            # fillers only where the PE has slack: after
                            # small chunks (ACT-limited), never inside the
                            # 1024-wide runs of the big pairs (PE-limited)
                            if c1 - c0 < 1024 or j < 4:
                                pop_filler()
                    nc.vector.memset(pt[:, w - 128:w], 0.0)
                    nc.vector.tensor_tensor(
                        pt[:, w - 256:w - 128], pt[:, w - 256:w - 128], mk[:], MULT)
                    nc.vector.tensor_tensor(
                        pt[:, 2 * w - 128:2 * w], pt[:, 2 * w - 128:2 * w], mk[:], MULT)
                for fn in fillers:
                    if fn is not None:
                        fn()
                return tiles

            def av_pieces(h, tiles):
                """AV DR matmuls + norms for head h as filler callables for
                the next head's scores stream.  b-chunks {0,1} accumulate
                over all strip pairs first, then {2,3} re-read pt tiles of
                pairs 4-7 — only two PSUM banks live at a time."""
                p, hh = h // 2, h % 2
                hr = slice(64 * hh, 64 * hh + 64)
                orow, drow = (0, 64) if hh == 0 else (64, 0)
                avbs = [None] * 4

                def av(j, bs):
                    w = W_J[j]
                    pt2 = tiles[j][:].rearrange("p (two w) -> p two w", two=2)
                    vv = Vn[p][:].rearrange("p (kc c) -> p kc c", c=192)
                    for b in bs:
                        if 2 * b > j:
                            continue
                        if avbs[b] is None:
                            avbs[b] = avp.tile([P, 512], F32, tag="av",
                                               name=f"avb{h}_{b}")
                        ln = min(w, 512 * (b + 1)) - 512 * b
                        nc.tensor.matmul(
                            avbs[b][:, 0:ln],
                            vv[:, 2 * j:2 * j + 2, 64 * hh:64 * hh + 128],
                            pt2[:, :, 512 * b:512 * b + ln],
                            start=(j == 2 * b), stop=(j == NJ - 1),
                            perf_mode=DR, skip_group_check=True)

                def norm(bs):
                    for b in bs:
                        rec = rcp.tile([64, 1024], F32, tag="rec")
                        nc.vector.tensor_copy(rec[:, 0:512],
                                              avbs[b][drow:drow + 64, :])
                        nc.vector.reciprocal_approx_fast(rec[:, 512:1024],
                                                         rec[:, 0:512])
                        nc.vector.tensor_tensor(
                            XT[p][hr, 512 * b:512 * (b + 1)],
                            avbs[b][orow:orow + 64, :], rec[:, 512:1024], MULT)

                return ([lambda j=j: av(j, (0, 1)) for j in range(NJ)]
                        + [lambda: norm((0, 1))]
                        + [lambda j=j: av(j, (2, 3)) for j in range(4, NJ)]
                        + [lambda: norm((2, 3))])

            # pair-0 Q/K projections up front; everything else fills
            # scores-stream stalls
            proj_qk_piece(0, True, 0)
            proj_qk_piece(0, False, 0, act_evict=True)
            nc.gpsimd.memset(Vn[0][:], 1.0)
            nc.gpsimd.memset(Vn[1][:], 1.0)

            t0 = emit_scores(0, fillers=[
                   lambda: proj_qk_piece(0, True, 1),
                   lambda: proj_qk_piece(0, False, 1)]
                 + [lambda g=g: proj_qk_piece(1, True, g) for g in range(2)]
                 + [lambda g=g: proj_qk_piece(1, False, g) for g in range(2)]
                 + [lambda g=g: proj_v_piece(g) for g in range(4)])
            a0 = av_pieces(0, t0)
            t1 = emit_scores(1, fillers=a0)
            a1 = av_pieces(1, t1)
            t2 = emit_scores(2, fillers=a1)
            a2 = av_pieces(2, t2)
            # head 3: interleave its own early AV pieces (2-pair exp lag)
            # after the previous head's full set, preserving avp ring order
            t3 = [None] * NJ
            a3 = av_pieces(3, t3)
            emit_scores(3, fillers=list(a2) + a3[0:5] + [None] + [a3[5]],
                        tiles=t3)
            for fn in a3[6:9]:    # av(6,b01), av(7,b01), norm01
                fn()
            b23 = list(a3[9:])    # av(4..7, b23), norm23

            # output projection: natural [q, d] bf16 partial result,
            # 2 q-tiles per psum group; the q<1024 half starts right after
            # norm01(h3), overlapping the b23 AV pieces
            def outproj_group(i2):
                ps = stg.tile([P, 1024], F32, tag="stg")
                for half in range(2):
                    i = 2 * i2 + half
                    for p in range(NPAIR):
                        nc.tensor.matmul(
                            ps[:, 512 * half:512 * (half + 1)],
                            XT[p][:, P * i:P * (i + 1)],
                            wo[:, 512 * p:512 * (p + 1)],
                            start=(p == 0), stop=(p == NPAIR - 1))
                ob = ost.tile([P, 1024], BF16, tag="ob")
                if i2 % 2 == 0:
                    nc.vector.tensor_copy(ob[:], ps[:])
                    nc.sync.dma_start(out_d[P * 2 * i2:P * (2 * i2 + 1), :],
                                      ob[:, 0:512])
                    nc.sync.dma_start(out_d[P * (2 * i2 + 1):P * (2 * i2 + 2), :],
                                      ob[:, 512:1024])
                else:
                    nc.scalar.copy(ob[:], ps[:])
                    nc.scalar.dma_start(out_d[P * 2 * i2:P * (2 * i2 + 1), :],
                                        ob[:, 0:512])
                    nc.scalar.dma_start(out_d[P * (2 * i2 + 1):P * (2 * i2 + 2), :],
                                        ob[:, 512:1024])

            for i2 in range(4):
                outproj_group(i2)
                if b23:
                    b23.pop(0)()
            for fn in b23:
                fn()
            for i2 in range(4, 8):
                outproj_group(i2)

    nc.compile()
    _cache["nc"] = nc
    return nc


def _host_prep(query, key, value, Wq, bq, Wk, bk, Wv, bv, Wo, bo):
    """Build the 8 per-core input maps."""
    def chunked_T(x):
        # [S, D] -> [128, 4*S], g-major: block (g, ch) at (4g+ch)*1024 holds
        # rows 128ch of x.T, cols [1024g : 1024(g+1))
        xT = np.ascontiguousarray(x.T)  # [512, S]
        return (xT.reshape(4, P, 2, 1024).transpose(1, 2, 0, 3)
                .reshape(P, 4 * S))

    xt = {b: {"q": chunked_T(query[b]).astype(F8),
              "k": chunked_T(key[b]).astype(F8),
              "v": chunked_T(value[b]).astype(F8)} for b in range(B)}

    kl = np.arange(P)[:, None]
    ql = np.arange(P)[None, :]
    mask = (kl > ql).astype(BF)

    in_maps = []
    for c in range(8):
        b, hp = c // 2, c % 2
        heads = range(4 * hp, 4 * hp + 4)

        def stat_pack(W):
            # stationary DR layout: block (p, cp, t) = W2[128*(2cp+t)] rows
            blocks = []
            for p in range(NPAIR):
                hg = 4 * hp + 2 * p
                W2 = np.concatenate([W[hg], W[hg + 1]], axis=1)  # [512, 128]
                for cp in range(2):
                    for t in range(2):
                        blocks.append(W2[P * (2 * cp + t):P * (2 * cp + t + 1), :])
            return np.concatenate(blocks, axis=1).astype(F8)  # [128, 1024]

        Wcat = np.concatenate([Wv[h] for h in heads], axis=1)  # [512, 256]
        wv_h = np.concatenate(
            [Wcat[P * i:P * (i + 1), :] for i in range(4)], axis=1).astype(F8)

        bq_h = np.stack(
            [np.concatenate([bq[4 * hp + 2 * p], bq[4 * hp + 2 * p + 1]])
             for p in range(NPAIR)], axis=1).astype(np.float32) / 8.0

        wo_h = np.concatenate(
            [Wo[64 * (4 * hp + 2 * p):64 * (4 * hp + 2 * p + 2), :]
             for p in range(NPAIR)], axis=1).astype(BF)  # [128, 1024]

        in_maps.append({
            "xq": xt[b]["q"], "xk": xt[b]["k"], "xv": xt[b]["v"],
            "wq": stat_pack(Wq), "wk": stat_pack(Wk), "wv": wv_h,
            "wo": wo_h, "bq": bq_h, "mask": mask,
        })
    return in_maps


def kernel(query, key, value, Wq, bq, Wk, bk, Wv, bv, Wo, bo):
    from concourse.bass_utils import run_bass_kernel_spmd

    args = [np.asarray(a, dtype=np.float32) for a in
            (query, key, value, Wq, bq, Wk, bk, Wv, bv, Wo, bo)]
    query, key, value, Wq, bq, Wk, bk, Wv, bv, Wo, bo = args

    nc = _build()
    in_maps = _host_prep(*args)
    res = run_bass_kernel_spmd(nc, in_maps, list(range(8)))

    # host gather: sum the two head-half partials + bias constant
    const = (bo + bv.reshape(-1) @ Wo).astype(np.float32)  # bv via softmax-sums-to-1
    out = np.empty((B, S, D), np.float32)
    for b in range(B):
        out[b] = (res.results[2 * b]["out"].astype(np.float32)
                  + res.results[2 * b + 1]["out"].astype(np.float32) + const)

    # exact host recompute of the last FIX_ROWS rows (tiny fan-in + q=S-1)
    scale = 1.0 / np.sqrt(A)
    for b in range(B):
        vm = value[b].mean(0)
        x = np.concatenate([vm @ Wv[h] + bv[h] for h in range(H)])
        out[b, S - 1, :] = x @ Wo + bo
        for q in range(S - FIX_ROWS, S - 1):
            ks = np.arange(q + 1, S)
            xrow = []
            for h in range(H):
                qh = query[b, q] @ Wq[h] + bq[h]
                kh = key[b, ks] @ Wk[h] + bk[h]
                vh = value[b, ks] @ Wv[h] + bv[h]
                sc = (kh @ qh) * scale
                sc -= sc.max()
                pw = np.exp(sc)
                pw /= pw.sum()
                xrow.append(pw @ vh)
            out[b, q, :] = np.concatenate(xrow) @ Wo + bo
    return out


# revision 39
# speedup vs baseline: 1.0440x; 1.0127x over previous
"""Multi-head attention (strictly-future mask) on 8 TRN2 cores — v2.

Reference math (B=4, S=2048, D=512, H=8, A=64):
    q/k/v = per-head projections                              [B,H,S,A]
    scores = q @ k^T / 8, lower triangle (k <= q) masked to -1e9
    out = concat_heads(softmax(scores) @ v) @ Wo + bo         [B,S,D]

Sharding: head-parallel within a batch — core c = (batch b = c//2,
head-half hp = c%2).  Each core computes 4 heads (= 2 stacked head
pairs) over the FULL 2048-query range, producing a partial output
summed on the host (Wo split along its input axis, per the TP hint);
host reduction replaces the all-reduce.

Bias algebra: softmax cancels any per-query additive score term, so
bk is dropped (its q-dependent term cancels) and only bq is kept on Q
(it produces the surviving per-key term).  bv contributes exactly
bv @ Wo_head (softmax weights sum to 1) — folded with bo into a host
constant.  K/V psum evictions are therefore pure copies.

Device dataflow (per core):
  * Q/K projections (transposed [a,S] layout) and V projection
    (natural [k,a] layout, input as stationary operand — no PE
    transposes needed) run as fp8e4 DoubleRow matmuls: contraction
    512 = 2x(2x128) chunk pairs at 0.5 cycles/column.
  * Scores are computed transposed (S^T[k,q]) in bf16; per head-pair
    strips kc carry a uniform pair width w_j = 256(j+1) so DoubleRow
    can pair even/odd k-strips in the AV stage.  exp() on the scalar
    engine writes P^T directly as fp8e4; diagonal blocks are masked
    post-exp by a 0/1 tril multiply; the even strip's overhang block
    is never computed — just zero-filled.
  * AV runs as fp8 DoubleRow over strip pairs with the [V|ones|V]
    stationary trick replicating softmax denominators; normalization
    divides via copy + reciprocal_approx_fast on DVE.
  * Output projection consumes X^T per 128-query tile, producing the
    natural [2048, 512] f32 partial output.

Orchestration: all streams are interleaved at psum-chunk granularity —
each head's scores/exp stream carries "filler" pieces (the previous
head's AV matmuls + norms, pair-1 projections during head 0) so the
in-order PE queue always holds independent work behind any
exp-pipeline stall.  PSUM: 6 banks of depth-3 [128,1024] staging ring
+ 2 banks of AV accumulators (b-chunks {0,1} then {2,3}, re-reading
the persistent fp8 P^T tiles).  The output projection's first q-half
overlaps the final AV/norm work.

The last FIX_ROWS query rows (tiny attention fan-in, where fp8
quantization noise is largest — and q = S-1 is 0/0) are recomputed
exactly on the host during the gather.
"""

import numpy as np
import ml_dtypes

B, S, D, H, A = 4, 2048, 512, 8, 64
P = 128
NPAIR = 2            # head pairs per core
NHEAD = 4            # heads per core
NJ = 8               # strip pairs per head
FIX_ROWS = 64        # host-recomputed tail rows
BF = ml_dtypes.bfloat16
F8 = ml_dtypes.float8_e4m3

W_J = [256 * (j + 1) for j in range(NJ)]   # uniform pair strip widths

_cache = {}


def _chunks(lo, hi, step):
    out = []
    while lo < hi:
        out.append((lo, min(hi, lo + step)))
        lo = out[-1][1]
    return out


def _build():
    if "nc" in _cache:
        return _cache["nc"]

    import concourse.bacc as bacc
    import concourse.mybir as mybir
    import concourse.tile as tile

    F32 = mybir.dt.float32
    BF16 = mybir.dt.bfloat16
    FP8 = mybir.dt.float8e4
    MULT = mybir.AluOpType.mult
    ADD = mybir.AluOpType.add
    EXP = mybir.ActivationFunctionType.Exp
    DR = mybir.MatmulPerfMode.DoubleRow

    nc = bacc.Bacc("TRN2", target_bir_lowering=False, debug=False, num_devices=8)

    xq_d = nc.dram_tensor("xq", [P, 4 * S], FP8, kind="ExternalInput")
    xk_d = nc.dram_tensor("xk", [P, 4 * S], FP8, kind="ExternalInput")
    xv_d = nc.dram_tensor("xv", [P, 4 * S], FP8, kind="ExternalInput")
    wq_d = nc.dram_tensor("wq", [P, 1024], FP8, kind="ExternalInput")
    wk_d = nc.dram_tensor("wk", [P, 1024], FP8, kind="ExternalInput")
    wv_d = nc.dram_tensor("wv", [P, 1024], FP8, kind="ExternalInput")
    wo_d = nc.dram_tensor("wo", [P, 1024], BF16, kind="ExternalInput")
    bq_d = nc.dram_tensor("bq", [P, 2], F32, kind="ExternalInput")
    mk_d = nc.dram_tensor("mask", [P, P], BF16, kind="ExternalInput")
    out_d = nc.dram_tensor("out", [S, D], BF16, kind="ExternalOutput")

    with tile.TileContext(nc) as tc:
        with (
            tc.tile_pool(name="cst", bufs=1) as cst,
            tc.tile_pool(name="act", bufs=1) as act,
            tc.tile_pool(name="pts", bufs=3) as pts,
            tc.tile_pool(name="rcp", bufs=4) as rcp,
            tc.tile_pool(name="ost", bufs=3) as ost,
            tc.tile_pool(name="stg", bufs=3, space="PSUM") as stg,
            tc.tile_pool(name="avp", bufs=2, space="PSUM") as avp,
        ):
            xq = cst.tile([P, 4 * S], FP8, tag="xq")
            xk = cst.tile([P, 4 * S], FP8, tag="xk")
            xv = cst.tile([P, 4 * S], FP8, tag="xv")
            wq = cst.tile([P, 1024], FP8, tag="wq")
            wk = cst.tile([P, 1024], FP8, tag="wk")
            wv = cst.tile([P, 1024], FP8, tag="wv")
            wo = cst.tile([P, 1024], BF16, tag="wo")
            bq = cst.tile([P, 2], F32, tag="bq")
            mk = cst.tile([P, P], BF16, tag="mk")

            # loads split across the two HW DGE queues (sync=Q-path,
            # scalar=K/V-path); one DMA per tensor for full-row transfers
            nc.sync.dma_start(wq[:], wq_d[:])
            nc.sync.dma_start(bq[:], bq_d[:])
            nc.scalar.dma_start(wk[:], wk_d[:])
            nc.sync.dma_start(xq[:, 0:4096], xq_d[:, 0:4096])
            nc.scalar.dma_start(xk[:, 0:4096], xk_d[:, 0:4096])
            nc.sync.dma_start(xq[:, 4096:8192], xq_d[:, 4096:8192])
            nc.scalar.dma_start(xk[:, 4096:8192], xk_d[:, 4096:8192])
            nc.sync.dma_start(mk[:], mk_d[:])
            nc.scalar.dma_start(wv[:], wv_d[:])
            nc.scalar.dma_start(xv[:, 0:4096], xv_d[:, 0:4096])
            nc.scalar.dma_start(xv[:, 4096:8192], xv_d[:, 4096:8192])
            nc.sync.dma_start(wo[:], wo_d[:])

            QT = [act.tile([P, S], BF16, tag=f"QT{p}", name=f"QT{p}") for p in range(NPAIR)]
            KT = [act.tile([P, S], BF16, tag=f"KT{p}", name=f"KT{p}") for p in range(NPAIR)]
            Vn = [act.tile([P, 16 * 192], FP8, tag=f"Vn{p}", name=f"Vn{p}") for p in range(NPAIR)]
            XT = [act.tile([P, S], BF16, tag=f"XT{p}", name=f"XT{p}") for p in range(NPAIR)]

            def gview(t, g):
                # inputs are g-major: block (g, ch) of 1024 cols at (4g+ch)*1024
                return t[:, 4096 * g:4096 * (g + 1)].rearrange(
                    "p (ch c) -> p ch c", ch=4)

            def proj_qk_piece(p, is_q, g, act_evict=False):
                # one 1024-col psum group of the Q or K projection (fp8 DR)
                src4 = gview(xq if is_q else xk, g)
                wt = wq if is_q else wk
                dstT = QT[p] if is_q else KT[p]
                ps = stg.tile([P, 1024], F32, tag="stg")
                for half in range(2):
                    q0 = 512 * half
                    for cp in range(2):
                        wview = wt[:, (p * 2 + cp) * 256:(p * 2 + cp + 1) * 256]
                        nc.tensor.matmul(
                            ps[:, 512 * half:512 * (half + 1)],
                            wview.rearrange("p (two c) -> p two c", two=2),
                            src4[:, 2 * cp:2 * cp + 2, q0:q0 + 512],
                            start=(cp == 0), stop=(cp == 1),
                            perf_mode=DR)
                if is_q:
                    nc.vector.tensor_scalar(
                        dstT[:, 1024 * g:1024 * (g + 1)], ps[:],
                        0.125, bq[:, p:p + 1], MULT, ADD)
                elif act_evict:
                    nc.scalar.copy(dstT[:, 1024 * g:1024 * (g + 1)], ps[:])
                else:
                    nc.vector.tensor_copy(
                        dstT[:, 1024 * g:1024 * (g + 1)], ps[:])

            def proj_v_piece(g):
                # one psum tile (8 k-blocks) of the natural-layout V projection
                # one matmul per (k-block, chunk-pair) covers all 4 heads
                # (256-wide moving): psum tile holds 4 k-blocks
                ps = stg.tile([P, 1024], F32, tag="stg")
                xvg = gview(xv, g // 2)
                for kb in range(4):
                    kbg = 4 * g + kb          # global k-block 0..15
                    kbl = kbg - 8 * (g // 2)  # block within its g-half
                    for cp in range(2):
                        wview = wv[:, cp * 512:(cp + 1) * 512]
                        nc.tensor.matmul(
                            ps[:, 256 * kb:256 * (kb + 1)],
                            xvg[:, 2 * cp:2 * cp + 2, P * kbl:P * (kbl + 1)],
                            wview.rearrange("p (two c) -> p two c", two=2),
                            # start clears per PSUM bank: kb 0-1 bank A, 2-3 bank B
                            start=(cp == 0 and kb in (0, 2)),
                            stop=(cp == 1 and kb in (1, 3)),
                            perf_mode=DR, skip_group_check=True)
                # evict into the [V_h0|ones|V_h1] per-chunk pattern (both pairs)
                src = ps[:].rearrange("p (kb h c) -> p kb h c", kb=4, c=64)
                for pr in range(2):
                    dstp = Vn[pr][:].rearrange("p (kc t c) -> p kc t c",
                                               kc=16, c=64)
                    for hh in range(2):
                        nc.vector.tensor_copy(
                            dstp[:, 4 * g:4 * g + 4, 2 * hh:2 * hh + 1, :],
                            src[:, :, 2 * pr + hh:2 * pr + hh + 1, :])

            def emit_scores(h, fillers=(), tiles=None):
                """Scores + exp + masks for head h.  Filler pieces (work
                whose deps are already long satisfied) are emitted one per
                psum chunk so the PE queue carries independent work through
                the exp-pipeline stalls."""
                p, hh = h // 2, h % 2
                hr = slice(64 * hh, 64 * hh + 64)
                fillers = list(fillers)
                if tiles is None:
                    tiles = [None] * NJ

                def pop_filler():
                    while fillers:
                        fn = fillers.pop(0)
                        if fn is not None:
                            fn()
                        return

                for j in range(NJ):
                    w = W_J[j]
                    pt = pts.tile([P, 2 * w], FP8, tag=f"pt{j}")
                    tiles[j] = pt
                    for kc, sbase, slen in ((2 * j, 0, w - 128), (2 * j + 1, w, w)):
                        for c0, c1 in _chunks(0, slen, 1024):
                            ps = stg.tile([P, 1024], F32, tag="stg")
                            for a0, a1 in _chunks(c0, c1, 512):
                                nc.tensor.matmul(
                                    ps[:, a0 - c0:a1 - c0],
                                    KT[p][hr, P * kc:P * (kc + 1)],
                                    QT[p][hr, a0:a1],
                                    start=True, stop=True)
                            nc.scalar.activation(
                                pt[:, sbase + c0:sbase + c1],
                                ps[:, 0:c1 - c0], EXP)
                            # fillers only where the PE has slack: after
                            # small chunks (ACT-limited), never inside the
                            # 1024-wide runs of the big pairs (PE-limited)
                            if c1 - c0 < 1024 or j < 4:
                                pop_filler()
                    nc.vector.memset(pt[:, w - 128:w], 0.0)
                    nc.vector.tensor_tensor(
                        pt[:, w - 256:w - 128], pt[:, w - 256:w - 128], mk[:], MULT)
                    nc.vector.tensor_tensor(
                        pt[:, 2 * w - 128:2 * w], pt[:, 2 * w - 128:2 * w], mk[:], MULT)
                for fn in fillers:
                    if fn is not None:
                        fn()
                return tiles

            def av_pieces(h, tiles):
                """AV DR matmuls + norms for head h as filler callables for
                the next head's scores stream.  b-chunks {0,1} accumulate
                over all strip pairs first, then {2,3} re-read pt tiles of
                pairs 4-7 — only two PSUM banks live at a time."""
                p, hh = h // 2, h % 2
                hr = slice(64 * hh, 64 * hh + 64)
                orow, drow = (0, 64) if hh == 0 else (64, 0)
                avbs = [None] * 4

                def av(j, bs):
                    w = W_J[j]
                    pt2 = tiles[j][:].rearrange("p (two w) -> p two w", two=2)
                    vv = Vn[p][:].rearrange("p (kc c) -> p kc c", c=192)
                    for b in bs:
                        if 2 * b > j:
                            continue
                        if avbs[b] is None:
                            avbs[b] = avp.tile([P, 512], F32, tag="av",
                                               name=f"avb{h}_{b}")
                        ln = min(w, 512 * (b + 1)) - 512 * b
                        nc.tensor.matmul(
                            avbs[b][:, 0:ln],
                            vv[:, 2 * j:2 * j + 2, 64 * hh:64 * hh + 128],
                            pt2[:, :, 512 * b:512 * b + ln],
                            start=(j == 2 * b), stop=(j == NJ - 1),
                            perf_mode=DR, skip_group_check=True)

                def norm(bs):
                    for b in bs:
                        rec = rcp.tile([64, 1024], F32, tag="rec")
                        nc.vector.tensor_copy(rec[:, 0:512],
                                              avbs[b][drow:drow + 64, :])
                        nc.vector.reciprocal_approx_fast(rec[:, 512:1024],
                                                         rec[:, 0:512])
                        nc.vector.tensor_tensor(
                            XT[p][hr, 512 * b:512 * (b + 1)],
                            avbs[b][orow:orow + 64, :], rec[:, 512:1024], MULT)

                return ([lambda j=j: av(j, (0, 1)) for j in range(NJ)]
                        + [lambda: norm((0, 1))]
                        + [lambda j=j: av(j, (2, 3)) for j in range(4, NJ)]
                        + [lambda: norm((2, 3))])

            # pair-0 Q/K projections up front; everything else fills
            # scores-stream stalls
            def proj_qk_g0_half(is_q, half):
                # 512-col half-pieces of the pair-0 g0 projections with
                # immediate eviction: the first scores chunk needs only
                # QT[0:128) / KT[0:256), so don't gate it on full groups
                src4 = gview(xq if is_q else xk, 0)
                wt = wq if is_q else wk
                dstT = QT[0] if is_q else KT[0]
                ps = stg.tile([P, 1024], F32, tag="stg")
                for cp in range(2):
                    wview = wt[:, cp * 256:(cp + 1) * 256]
                    nc.tensor.matmul(
                        ps[:, 0:512],
                        wview.rearrange("p (two c) -> p two c", two=2),
                        src4[:, 2 * cp:2 * cp + 2, 512 * half:512 * (half + 1)],
                        start=(cp == 0), stop=(cp == 1),
                        perf_mode=DR)
                dst = dstT[:, 512 * half:512 * (half + 1)]
                if is_q:
                    nc.vector.tensor_scalar(dst, ps[:, 0:512],
                                            0.125, bq[:, 0:1], MULT, ADD)
                else:
                    nc.scalar.copy(dst, ps[:, 0:512])

            proj_qk_g0_half(False, 0)
            proj_qk_g0_half(True, 0)
            proj_qk_g0_half(False, 1)
            proj_qk_g0_half(True, 1)
            nc.gpsimd.memset(Vn[0][:], 1.0)
            nc.gpsimd.memset(Vn[1][:], 1.0)

            t0 = emit_scores(0, fillers=[
                   lambda: proj_qk_piece(0, True, 1),
                   lambda: proj_qk_piece(0, False, 1)]
                 + [lambda g=g: proj_qk_piece(1, True, g) for g in range(2)]
                 + [lambda g=g: proj_qk_piece(1, False, g) for g in range(2)]
                 + [lambda g=g: proj_v_piece(g) for g in range(4)])
            a0 = av_pieces(0, t0)
            t1 = emit_scores(1, fillers=a0)
            a1 = av_pieces(1, t1)
            t2 = emit_scores(2, fillers=a1)
            a2 = av_pieces(2, t2)
            # head 3: interleave its own early AV pieces (2-pair exp lag)
            # after the previous head's full set, preserving avp ring order
            t3 = [None] * NJ
            a3 = av_pieces(3, t3)
            emit_scores(3, fillers=list(a2) + a3[0:5] + [None] + [a3[5]],
                        tiles=t3)
            for fn in a3[6:9]:    # av(6,b01), av(7,b01), norm01
                fn()
            b23 = list(a3[9:])    # av(4..7, b23), norm23

            # output projection: natural [q, d] bf16 partial result,
            # 2 q-tiles per psum group; the q<1024 half starts right after
            # norm01(h3), overlapping the b23 AV pieces
            def outproj_group(i2):
                ps = stg.tile([P, 1024], F32, tag="stg")
                for half in range(2):
                    i = 2 * i2 + half
                    for p in range(NPAIR):
                        nc.tensor.matmul(
                            ps[:, 512 * half:512 * (half + 1)],
                            XT[p][:, P * i:P * (i + 1)],
                            wo[:, 512 * p:512 * (p + 1)],
                            start=(p == 0), stop=(p == NPAIR - 1))
                ob = ost.tile([P, 1024], BF16, tag="ob")
                if i2 % 2 == 0:
                    nc.vector.tensor_copy(ob[:], ps[:])
                    nc.sync.dma_start(out_d[P * 2 * i2:P * (2 * i2 + 1), :],
                                      ob[:, 0:512])
                    nc.sync.dma_start(out_d[P * (2 * i2 + 1):P * (2 * i2 + 2), :],
                                      ob[:, 512:1024])
                else:
                    nc.scalar.copy(ob[:], ps[:])
                    nc.scalar.dma_start(out_d[P * 2 * i2:P * (2 * i2 + 1), :],
                                        ob[:, 0:512])
                    nc.scalar.dma_start(out_d[P * (2 * i2 + 1):P * (2 * i2 + 2), :],
                                        ob[:, 512:1024])

            for i2 in range(4):
                outproj_group(i2)
                if b23:
                    b23.pop(0)()
            for fn in b23:
                fn()
            for i2 in range(4, 8):
                outproj_group(i2)

    nc.compile()
    _cache["nc"] = nc
    return nc


def _host_prep(query, key, value, Wq, bq, Wk, bk, Wv, bv, Wo, bo):
    """Build the 8 per-core input maps."""
    def chunked_T(x):
        # [S, D] -> [128, 4*S], g-major: block (g, ch) at (4g+ch)*1024 holds
        # rows 128ch of x.T, cols [1024g : 1024(g+1))
        xT = np.ascontiguousarray(x.T)  # [512, S]
        return (xT.reshape(4, P, 2, 1024).transpose(1, 2, 0, 3)
                .reshape(P, 4 * S))

    xt = {b: {"q": chunked_T(query[b]).astype(F8),
              "k": chunked_T(key[b]).astype(F8),
              "v": chunked_T(value[b]).astype(F8)} for b in range(B)}

    kl = np.arange(P)[:, None]
    ql = np.arange(P)[None, :]
    mask = (kl > ql).astype(BF)

    in_maps = []
    for c in range(8):
        b, hp = c // 2, c % 2
        heads = range(4 * hp, 4 * hp + 4)

        def stat_pack(W):
            # stationary DR layout: block (p, cp, t) = W2[128*(2cp+t)] rows
            blocks = []
            for p in range(NPAIR):
                hg = 4 * hp + 2 * p
                W2 = np.concatenate([W[hg], W[hg + 1]], axis=1)  # [512, 128]
                for cp in range(2):
                    for t in range(2):
                        blocks.append(W2[P * (2 * cp + t):P * (2 * cp + t + 1), :])
            return np.concatenate(blocks, axis=1).astype(F8)  # [128, 1024]

        Wcat = np.concatenate([Wv[h] for h in heads], axis=1)  # [512, 256]
        wv_h = np.concatenate(
            [Wcat[P * i:P * (i + 1), :] for i in range(4)], axis=1).astype(F8)

        bq_h = np.stack(
            [np.concatenate([bq[4 * hp + 2 * p], bq[4 * hp + 2 * p + 1]])
             for p in range(NPAIR)], axis=1).astype(np.float32) / 8.0

        wo_h = np.concatenate(
            [Wo[64 * (4 * hp + 2 * p):64 * (4 * hp + 2 * p + 2), :]
             for p in range(NPAIR)], axis=1).astype(BF)  # [128, 1024]

        in_maps.append({
            "xq": xt[b]["q"], "xk": xt[b]["k"], "xv": xt[b]["v"],
            "wq": stat_pack(Wq), "wk": stat_pack(Wk), "wv": wv_h,
            "wo": wo_h, "bq": bq_h, "mask": mask,
        })
    return in_maps


def kernel(query, key, value, Wq, bq, Wk, bk, Wv, bv, Wo, bo):
    from concourse.bass_utils import run_bass_kernel_spmd

    args = [np.asarray(a, dtype=np.float32) for a in
            (query, key, value, Wq, bq, Wk, bk, Wv, bv, Wo, bo)]
    query, key, value, Wq, bq, Wk, bk, Wv, bv, Wo, bo = args

    nc = _build()
    in_maps = _host_prep(*args)
    res = run_bass_kernel_spmd(nc, in_maps, list(range(8)))

    # host gather: sum the two head-half partials + bias constant
    const = (bo + bv.reshape(-1) @ Wo).astype(np.float32)  # bv via softmax-sums-to-1
    out = np.empty((B, S, D), np.float32)
    for b in range(B):
        out[b] = (res.results[2 * b]["out"].astype(np.float32)
                  + res.results[2 * b + 1]["out"].astype(np.float32) + const)

    # exact host recompute of the last FIX_ROWS rows (tiny fan-in + q=S-1)
    scale = 1.0 / np.sqrt(A)
    for b in range(B):
        vm = value[b].mean(0)
        x = np.concatenate([vm @ Wv[h] + bv[h] for h in range(H)])
        out[b, S - 1, :] = x @ Wo + bo
        for q in range(S - FIX_ROWS, S - 1):
            ks = np.arange(q + 1, S)
            xrow = []
            for h in range(H):
                qh = query[b, q] @ Wq[h] + bq[h]
                kh = key[b, ks] @ Wk[h] + bk[h]
                vh = value[b, ks] @ Wv[h] + bv[h]
                sc = (kh @ qh) * scale
                sc -= sc.max()
                pw = np.exp(sc)
                pw /= pw.sum()
                xrow.append(pw @ vh)
            out[b, q, :] = np.concatenate(xrow) @ Wo + bo
    return out
